# revision 1
# baseline (speedup 1.0000x reference)
"""Multi-head attention (B=4, S=2048, E=512, H=8) on 8 TRN2 NeuronCores.

Sharding: (batch, head-group) — core c handles batch c//2 and heads
[4*(c%2), 4*(c%2)+4). Each core computes QKV projections for its 4 heads,
flash-style attention (scores kept transposed on-chip, never spilled to HBM),
and a partial output projection over its 256 attention dims. Host sums the
two partials per batch and adds the output bias.

Per-core device program (S=2048, E=512, D=64, 4 heads):
  xT     = transpose(x_b)                      via PE transpose, [E, S] in SBUF
  QT/KT  = W @ xT (+bias)                      [d(head pair), S] layout
  V      = xT.T @ WvT (+bias), ones column     [S, d+1] layout per head
  scoresT= KT.T @ QT per (sk,sq) tile          PSUM, exp on ACT -> PT (SBUF)
  outT  += V_ext.T @ PT                        [d+1, sq] accum; row d = sums
  out    = outT[:d] * (1/sums) broadcast       PE outer-product broadcast
  y_part = out.T @ WoT                         [S, E] partial, DMA to HBM

All matmuls run as float32r (full fp32 storage, relaxed multiply).
"""

import os
from contextlib import ExitStack

import numpy as np
import ml_dtypes

import concourse.bacc as bacc
import concourse.mybir as mybir
import concourse.tile as tile
from concourse.masks import make_identity

F32 = mybir.dt.float32
F32R = mybir.dt.float32r
F16 = mybir.dt.float16
BF16 = mybir.dt.bfloat16
EXP = mybir.ActivationFunctionType.Exp

E = 512       # embed dim
D = 64        # head dim
HPC = 4       # heads per core
NE = E // 128  # e-tiles (4)


def build(S=2048):
    """Build the per-core SPMD program. Returns compiled Bacc."""
    nS = S // 128   # 128-wide s-chunks
    nSq = S // 512  # 512-wide s-chunks
    nc = bacc.Bacc(None, target_bir_lowering=False, debug=False)

    xb = nc.dram_tensor("xb", [S, E], F32, kind="ExternalInput")
    wqT_d = nc.dram_tensor("wqT", [E, 256], BF16, kind="ExternalInput")
    wkT_d = nc.dram_tensor("wkT", [E, 256], BF16, kind="ExternalInput")
    wvT_d = nc.dram_tensor("wvT", [E, 256], BF16, kind="ExternalInput")
    woT_d = nc.dram_tensor("woT", [256, E], BF16, kind="ExternalInput")
    bq_d = nc.dram_tensor("bq", [2, 128, 1], F32, kind="ExternalInput")
    bk_d = nc.dram_tensor("bk", [2, 2, 128, 1], F32, kind="ExternalInput")
    mk_d = nc.dram_tensor("maskd", [2, 128, 1], F32, kind="ExternalInput")
    bv_d = nc.dram_tensor("bv", [128, 256], F32, kind="ExternalInput")
    yp = nc.dram_tensor("yp", [S, E], F32, kind="ExternalOutput")
    yp1 = nc.dram_tensor("yp1", [S, E], F32, kind="ExternalOutput")

    xb_r = xb[:].rearrange("(n p) e -> n p e", p=128)
    yp_r = yp[:].rearrange("(n p) e -> n p e", p=128)
    yp1_r = yp1[:].rearrange("(n p) e -> n p e", p=128)

    with tile.TileContext(nc) as tc, ExitStack() as ctx:
        const = ctx.enter_context(tc.tile_pool(name="const", bufs=1))
        wpool = ctx.enter_context(tc.tile_pool(name="weights", bufs=1))
        big = ctx.enter_context(tc.tile_pool(name="big", bufs=1))
        xstage = ctx.enter_context(tc.tile_pool(name="xstage", bufs=3))
        ptpool = ctx.enter_context(tc.tile_pool(name="pt", bufs=4))
        smpool = ctx.enter_context(tc.tile_pool(name="small", bufs=2))
        bcpool = ctx.enter_context(tc.tile_pool(name="bcs", bufs=2))
        ypool = ctx.enter_context(tc.tile_pool(name="ysb", bufs=3))
        ps1 = ctx.enter_context(tc.tile_pool(name="ps1", bufs=2, space="PSUM"))
        ps_acc = ctx.enter_context(tc.tile_pool(name="psacc", bufs=2, space="PSUM"))
        ps_y = ctx.enter_context(tc.tile_pool(name="psy", bufs=1, space="PSUM"))
        ps_bc = ctx.enter_context(tc.tile_pool(name="psbc", bufs=1, space="PSUM"))

        ident = const.tile([128, 128], F32)
        make_identity(nc, ident[:])
        ones_pf = const.tile([128, 64], F32)
        nc.vector.memset(ones_pf[:], 1.0)
        ones64 = const.tile([1, 64], BF16)
        nc.vector.tensor_copy(ones64[:], ones_pf[0:1, :])
        bq_sb = [const.tile([128, 1], F32, name=f"bq{i}", tag=f"bq{i}") for i in range(2)]
        bk_sb = [[const.tile([128, 1], F32, name=f"bk{i}_{j}", tag=f"bk{i}_{j}")
                  for j in range(2)] for i in range(2)]
        mk_sb = [const.tile([128, 1], F32, name=f"mk{j}", tag=f"mk{j}") for j in range(2)]
        bv_sb = const.tile([128, 256], F32)
        for hp in range(2):
            nc.sync.dma_start(bq_sb[hp][:], bq_d[hp])
            for hq in range(2):
                nc.sync.dma_start(bk_sb[hp][hq][:], bk_d[hp, hq])
        for hq in range(2):
            nc.sync.dma_start(mk_sb[hq][:], mk_d[hq])
        nc.sync.dma_start(bv_sb[:], bv_d[:])

        # weights: wqT/wkT as [128, et*256 + hp*128 + d'] (stationary slices)
        # wvT as [128, et*256 + (h,d)] (moving), woT as [128, ct*512 + e]
        wq_sb = wpool.tile([128, NE * 256], BF16)
        wk_sb = wpool.tile([128, NE * 256], BF16)
        wv_sb = wpool.tile([128, NE * 256], BF16)
        wo_sb = wpool.tile([128, 2 * E], BF16)
        wq_r = wqT_d[:].rearrange("(t p) c -> t p c", p=128)
        wk_r = wkT_d[:].rearrange("(t p) c -> t p c", p=128)
        wv_r = wvT_d[:].rearrange("(t p) c -> t p c", p=128)
        wo_r = woT_d[:].rearrange("(t p) c -> t p c", p=128)
        for t in range(NE):
            nc.sync.dma_start(wq_sb[:, t * 256:(t + 1) * 256], wq_r[t])
            nc.sync.dma_start(wk_sb[:, t * 256:(t + 1) * 256], wk_r[t])
            nc.sync.dma_start(wv_sb[:, t * 256:(t + 1) * 256], wv_r[t])
        for t in range(2):
            nc.sync.dma_start(wo_sb[:, t * E:(t + 1) * E], wo_r[t])

        # big SBUF tensors
        xT_sb = big.tile([128, NE * S], BF16)     # (e%128, et*S + s)
        qT_sb = big.tile([128, 2 * S], BF16)      # (h'*64+d, hp*S + s)
        kp_sb = big.tile([128, HPC * S], BF16)  # per-head, other pair-half zeroed
        v_sb = big.tile([128, HPC * nS * 65], BF16)  # (s%128, h*(nS*65)+c*65+d)
        aoT_sb = big.tile([128, 2 * S], BF16)     # (h'*64+d, ct*S + s)

        # ---- projection emitters ----
        def qk_group(hp, w_sb, j, pool=None, ptag="ps1"):
            pool = pool or ps1
            pp = pool.tile([128, 512], F32, name=f"pp{hp}_{1 if w_sb is wq_sb else 0}_{j}", tag=ptag)
            for et in range(NE):
                nc.tensor.matmul(
                    pp[:],
                    lhsT=w_sb[:, et * 256 + hp * 128: et * 256 + (hp + 1) * 128],
                    rhs=xT_sb[:, et * S + j * 512: et * S + (j + 1) * 512],
                    start=(et == 0), stop=(et == NE - 1))
            if w_sb is wq_sb:
                # fold the 1/sqrt(D) softmax scale into Q (and its bias)
                nc.vector.tensor_scalar(
                    qT_sb[:, hp * S + j * 512: hp * S + (j + 1) * 512],
                    pp[:], 0.125, bq_sb[hp][:],
                    op0=mybir.AluOpType.mult, op1=mybir.AluOpType.add)
            else:
                # zero-padded per-head KT: mask kills the other head's rows
                for hq in range(2):
                    h = 2 * hp + hq
                    nc.vector.tensor_scalar(
                        kp_sb[:, h * S + j * 512: h * S + (j + 1) * 512],
                        pp[:], mk_sb[hq][:], bk_sb[hp][hq][:],
                        op0=mybir.AluOpType.mult, op1=mybir.AluOpType.add)

        # ---- Phases A/B/C interleaved per s-chunk ----
        # transpose chunk i -> V(i) immediately; after chunks 4j+3: K(j), Q(j).
        xT_r = xT_sb[:].rearrange("p (t s) -> p t s", t=NE)
        v_view = v_sb[:].rearrange("p (h s) -> p h s", h=HPC)

        def tr_chunk(i):
            xt = xstage.tile([128, E], F32, tag="xs")
            nc.sync.dma_start(xt[:], xb_r[i])
            tp = ps1.tile([128, E], F32, tag="ps1")
            for et in range(NE):
                nc.tensor.transpose(tp[:, et * 128:(et + 1) * 128],
                                    xt[:, et * 128:(et + 1) * 128], ident[:])
            nc.vector.tensor_copy(
                xT_r[:, :, i * 128:(i + 1) * 128],
                tp[:].rearrange("p (t s) -> p t s", t=NE))

        def v_group(i):
            vp = ps1.tile([128, 512], F32, name=f"vp{i}", tag="ps1")
            vps = vp[:, 0:256]
            for et in range(NE):
                nc.tensor.matmul(
                    vps,
                    lhsT=xT_sb[:, et * S + i * 128: et * S + (i + 1) * 128],
                    rhs=wv_sb[:, et * 256:(et + 1) * 256],
                    start=(et == 0), stop=(et == NE - 1))
            nc.vector.tensor_add(
                v_view[:, :, i * 65: i * 65 + 64],
                vps.rearrange("p (h d) -> p h d", h=HPC),
                bv_sb[:].rearrange("p (h d) -> p h d", h=HPC))

        # block-interleave: transposes in blocks, projection groups between
        # blocks (their xT inputs are a block behind, so PE never waits on DVE)
        nc.vector.tensor_copy(
            v_sb[:].rearrange("p (g d) -> p g d", d=65)[:, :, 64],
            ones_pf[:, :HPC * nS])
        c4 = nS // 4
        if nS < 16:  # small-S (sim) fallback: strictly phase-ordered
            for i in range(nS):
                tr_chunk(i)
            for j in range(nSq):
                qk_group(0, wk_sb, j)
                qk_group(0, wq_sb, j)
            for i in range(nS):
                v_group(i)
            emit_fancy = False
        else:
            emit_fancy = True
        for i in range(0, 2 * c4 if emit_fancy else 0):
            tr_chunk(i)
        if emit_fancy:
            qk_group(0, wk_sb, 0)
            for i in range(2 * c4, 3 * c4):
                tr_chunk(i)
            qk_group(0, wq_sb, 0)
            qk_group(0, wk_sb, 1)
            for i in range(3 * c4, nS):
                tr_chunk(i)
            qk_group(0, wq_sb, 1)
            for i in range(0, nS // 2):
                v_group(i)
            for j in range(2, nSq):
                qk_group(0, wk_sb, j)
                qk_group(0, wq_sb, j)
            for i in range(nS // 2, nS):
                v_group(i)

        def out_proj_chunk(ct, i, dst_r):
            yps = ps_y.tile([128, 512], F32, name=f"yps{ct}_{i}", tag="yp")
            nc.tensor.matmul(
                yps[:],
                lhsT=aoT_sb[:, ct * S + i * 128: ct * S + (i + 1) * 128],
                rhs=wo_sb[:, ct * E:(ct + 1) * E],
                start=True, stop=True)
            ys = ypool.tile([128, E], F32, name=f"ys{ct}_{i}", tag="ys")
            nc.vector.tensor_copy(ys[:], yps[:])
            nc.sync.dma_start(dst_r[i], ys[:])

        # ---- Phase D: attention per head ----
        nH2 = S // 1024  # sq halves

        def attention(hp, iter_filler=None):
            for hq in range(2):
                h = 2 * hp + hq
                for half in range(nH2):
                    sq0 = hp * S + half * 1024
                    accs = [ps_acc.tile([65, 512], F32, name=f"acc{h}_{half}_{j}",
                                        tag="acc") for j in range(2)]
                    for i in range(nS):
                        sc = ps1.tile([128, 1024], F32, tag="ps1")
                        for j2 in range(2):
                            nc.tensor.matmul(
                                sc[:, j2 * 512:(j2 + 1) * 512],
                                lhsT=kp_sb[:, h * S + i * 128: h * S + (i + 1) * 128],
                                rhs=qT_sb[:, sq0 + j2 * 512: sq0 + (j2 + 1) * 512],
                                start=True, stop=True)
                        pt = ptpool.tile([128, 1024], BF16, tag="pt")
                        nc.scalar.activation(pt[:], sc[:], EXP)
                        for j2 in range(2):
                            nc.tensor.matmul(
                                accs[j2][:],
                                lhsT=v_sb[:, h * nS * 65 + i * 65: h * nS * 65 + i * 65 + 65],
                                rhs=pt[:, j2 * 512:(j2 + 1) * 512],
                                start=(i == 0), stop=(i == nS - 1),
                                skip_group_check=True)
                        if iter_filler is not None:
                            iter_filler(hq, half, i)
                    # normalize: aoT[dq, s] = accs[:64] * (1/sums) bcast.
                    # Copy out of psum first so the acc banks release fast.
                    for j2 in range(2):
                        ssb = smpool.tile([1, 512], F32, tag="ssb")
                        nc.vector.tensor_copy(ssb[:], accs[j2][64:65, :])
                        aou = bcpool.tile([64, 512], F32, tag="aou")
                        nc.vector.tensor_copy(aou[:], accs[j2][0:64, :])
                        rsb = smpool.tile([1, 512], F32, tag="rsb")
                        nc.vector.reciprocal_approx_fast(rsb[:], ssb[:])
                        rf = smpool.tile([1, 512], BF16, tag="rf")
                        nc.vector.tensor_copy(rf[:], rsb[:])
                        bc = ps_bc.tile([64, 512], F32, tag="bc")
                        nc.tensor.matmul(bc[:], lhsT=ones64[:],
                                         rhs=rf[:], start=True, stop=True)
                        nc.vector.tensor_mul(
                            aoT_sb[64 * hq: 64 * hq + 64,
                                   sq0 + j2 * 512: sq0 + (j2 + 1) * 512],
                            aou[:], bc[:])

        # hp1 projections stream through attention(0)'s per-iteration PE slack,
        # using the out-proj psum bank (idle during attention(0)).
        qk1 = [(w, j) for w in (wk_sb, wq_sb) for j in range(nSq)]
        n_it0 = 2 * nH2 * nS  # iterations in attention(0)
        stride0 = max(1, n_it0 // (len(qk1) + 1))
        emitted = {"k": 0, "it": 0}

        def att0_filler(hq, half, i):
            emitted["it"] += 1
            if emitted["it"] % stride0 == 0 and emitted["k"] < len(qk1):
                w, j = qk1[emitted["k"]]
                qk_group(1, w, j, pool=ps_y, ptag="yp")
                emitted["k"] += 1

        attention(0, att0_filler)
        for k in range(emitted["k"], len(qk1)):
            w, j = qk1[k]
            qk_group(1, w, j)

        # out_proj(0) chunks stream through attention(1) h2's iterations;
        # out_proj(1) chunks 0..nS/2-1 through h3-half1; the rest trail.
        op_plan = {}
        for c in range(nS):           # ct0 chunks during hq=0 (head 2)
            half, it = divmod(c * (nH2 * nS) // nS, nS)
            op_plan[(0, half, it)] = (0, c)
        if nH2 >= 2:
            for c in range(nS // 2):  # ct1 low chunks during h3's LAST half
                op_plan[(1, nH2 - 1, (c * 2) % nS)] = (1, c)
        op1_tail_start = nS // 2 if nH2 >= 2 else 0

        def att1_filler(hq, half, i):
            job = op_plan.get((hq, half, i))
            if job is not None:
                ct, c = job
                out_proj_chunk(ct, c, yp_r if ct == 0 else yp1_r)

        attention(1, att1_filler)
        for c in range(op1_tail_start, nS):
            out_proj_chunk(1, c, yp1_r)

    nc.compile()
    return nc


def make_in_maps(x, w_qkv, b_qkv, w_out):
    """Build the 8 per-core input dicts from full inputs."""
    in_maps = []
    for c in range(8):
        b, hg = c // 2, c % 2
        r0 = hg * 256
        bk = b_qkv[512 + r0:512 + r0 + 256].reshape(2, 128, 1)
        maskd = np.zeros((2, 128, 1), dtype=np.float32)
        maskd[0, :64] = 1.0
        maskd[1, 64:] = 1.0
        bk_pad = (bk[:, None, :, :] * maskd[None, :, :, :]).astype(np.float32)
        wq = w_qkv[r0:r0 + 256, :]
        wk = w_qkv[512 + r0:512 + r0 + 256, :]
        wv = w_qkv[1024 + r0:1024 + r0 + 256, :]
        in_maps.append({
            "xb": np.ascontiguousarray(x[b]),
            "wqT": np.ascontiguousarray(wq.T.astype(ml_dtypes.bfloat16)),
            "wkT": np.ascontiguousarray(wk.T.astype(ml_dtypes.bfloat16)),
            "wvT": np.ascontiguousarray(wv.T.astype(ml_dtypes.bfloat16)),
            "woT": np.ascontiguousarray(w_out[:, r0:r0 + 256].T.astype(ml_dtypes.bfloat16)),
            "bq": np.ascontiguousarray(b_qkv[r0:r0 + 256].reshape(2, 128, 1) * 0.125),
            "bk": np.ascontiguousarray(bk_pad),
            "maskd": np.ascontiguousarray(maskd),
            "bv": np.ascontiguousarray(
                np.tile(b_qkv[1024 + r0:1024 + r0 + 256][None, :], (128, 1))),
        })
    return in_maps


_cached_nc = None
last_exec_time_ns = None
last_result = None


def kernel(x, w_qkv, b_qkv, w_out, b_out):
    global _cached_nc, last_exec_time_ns, last_result
    from concourse.bass_utils import run_bass_kernel_spmd

    x = np.asarray(x, dtype=np.float32)
    w_qkv = np.asarray(w_qkv, dtype=np.float32)
    b_qkv = np.asarray(b_qkv, dtype=np.float32)
    w_out = np.asarray(w_out, dtype=np.float32)
    b_out = np.asarray(b_out, dtype=np.float32)
    B, S, _ = x.shape

    if _cached_nc is None:
        _cached_nc = build(S)
    nc = _cached_nc

    in_maps = make_in_maps(x, w_qkv, b_qkv, w_out)
    trace = bool(os.environ.get("BASS_KERNEL_TRACE"))
    r = run_bass_kernel_spmd(nc, in_maps, core_ids=list(range(8)), trace=trace)
    last_exec_time_ns = r.exec_time_ns
    last_result = r

    y = np.empty((B, S, E), dtype=np.float32)
    for b in range(B):
        y[b] = (r.results[2 * b]["yp"] + r.results[2 * b]["yp1"]
                + r.results[2 * b + 1]["yp"] + r.results[2 * b + 1]["yp1"] + b_out)
    return y



# revision 11
# speedup vs baseline: 1.0503x; 1.0503x over previous
"""Multi-head attention (B=4, S=2048, E=512, H=8) on 8 TRN2 NeuronCores.

Sharding: (batch, head-pair) - core c handles batch c//2 and heads
[4*(c%2), 4*(c%2)+4). Each core computes QKV projections for its 4 heads,
flash-style attention, and a partial output projection over its 256
attention dims. Host sums the two bf16 partials per batch + bias.

v2 schedule (vs v1): attention starts ~7us in (right after K j=0 and
Q j=0,1); every other projection / transpose / out-proj chunk is a PE
filler inside the EXP-paced attention loop. The AV matmul runs one
iteration behind its EXP so the in-order PE queue never blocks on the
Scalar engine. Scores use 64-partition matmuls (no zero-padded K).
x loads are split across the two HW DMA queues (sync + scalar) and
issued first; output partials are bf16.

Per-iteration steady state (1 of 128):
  sc   = K_h[:, i128].T @ Q_h[:, half]     2x [128,512] psum matmuls
  pt   = exp(sc)                           Scalar ACT, [128,1024] -> bf16
  acc += V_ext(i-1).T @ pt(i-1)            2x [65,512] accumulating (delayed)
  + filler (transpose / K,Q,V proj group / out-proj chunk)
"""

import os
from collections import defaultdict
from contextlib import ExitStack

import numpy as np
import ml_dtypes

import concourse.bacc as bacc
import concourse.mybir as mybir
import concourse.tile as tile
from concourse.masks import make_identity

F32 = mybir.dt.float32
BF16 = mybir.dt.bfloat16
EXP = mybir.ActivationFunctionType.Exp
COPY = mybir.ActivationFunctionType.Copy

E = 512       # embed dim
D = 64        # head dim
HPC = 4       # heads per core
NE = E // 128  # e-tiles (4)


def build(S=2048):
    """Build the per-core SPMD program. Returns compiled Bacc."""
    nS = S // 128    # 128-wide s-chunks
    nSq = S // 512   # 512-wide s-chunks (q-proj groups)
    W = 1024 if S >= 1024 else S   # attention q-tile width
    nH2 = S // W     # q halves
    JW = W // 512    # 512-wide subtiles per q-tile
    fancy = (nS == 16 and nH2 == 2)

    nc = bacc.Bacc(None, target_bir_lowering=False, debug=False)

    xb = nc.dram_tensor("xb", [S, E], F32, kind="ExternalInput")
    wqT_d = nc.dram_tensor("wqT", [E, 256], BF16, kind="ExternalInput")
    wkT_d = nc.dram_tensor("wkT", [E, 256], BF16, kind="ExternalInput")
    wvT_d = nc.dram_tensor("wvT", [E, 256], BF16, kind="ExternalInput")
    woT_d = nc.dram_tensor("woT", [256, E], BF16, kind="ExternalInput")
    bq_d = nc.dram_tensor("bq", [2, 128, 1], F32, kind="ExternalInput")
    bk_d = nc.dram_tensor("bk", [2, 128, 1], F32, kind="ExternalInput")
    bv_d = nc.dram_tensor("bv", [128, 256], F32, kind="ExternalInput")
    yp0 = nc.dram_tensor("yp0", [S, E], BF16, kind="ExternalOutput")
    yp1 = nc.dram_tensor("yp1", [S, E], BF16, kind="ExternalOutput")

    xb_r = xb[:].rearrange("(n p) e -> n p e", p=128)
    yp_rs = [yp0[:].rearrange("(n p) e -> n p e", p=128),
             yp1[:].rearrange("(n p) e -> n p e", p=128)]

    with tile.TileContext(nc) as tc, ExitStack() as ctx:
        const = ctx.enter_context(tc.tile_pool(name="const", bufs=1))
        wpool = ctx.enter_context(tc.tile_pool(name="weights", bufs=1))
        big = ctx.enter_context(tc.tile_pool(name="big", bufs=1))
        xcpool = ctx.enter_context(tc.tile_pool(name="xc", bufs=3))
        ptpool = ctx.enter_context(tc.tile_pool(name="pt", bufs=4))
        smpool = ctx.enter_context(tc.tile_pool(name="small", bufs=2))
        bcpool = ctx.enter_context(tc.tile_pool(name="bcs", bufs=2))
        ypool = ctx.enter_context(tc.tile_pool(name="ysb", bufs=3))
        ps_sc = ctx.enter_context(tc.tile_pool(name="pssc", bufs=2, space="PSUM"))
        ps_acc = ctx.enter_context(tc.tile_pool(name="psacc", bufs=1, space="PSUM"))
        ps_util = ctx.enter_context(tc.tile_pool(name="psutil", bufs=2, space="PSUM"))

        # ---- DMA: x first, on both HW queues; weights interleaved ----
        xraw = big.tile([128, nS * 512], F32)
        wq_sb = wpool.tile([128, NE * 256], BF16)
        wk_sb = wpool.tile([128, NE * 256], BF16)
        wv_sb = wpool.tile([128, NE * 256], BF16)
        wo_sb = wpool.tile([128, 2 * E], BF16)
        bq_sb = [const.tile([128, 1], F32, name=f"bq{i}", tag=f"bq{i}") for i in range(2)]
        bk_sb = [const.tile([128, 1], F32, name=f"bk{i}", tag=f"bk{i}") for i in range(2)]
        bv_sb = const.tile([128, 256], F32)
        wq_r = wqT_d[:].rearrange("(t p) c -> t p c", p=128)
        wk_r = wkT_d[:].rearrange("(t p) c -> t p c", p=128)
        wv_r = wvT_d[:].rearrange("(t p) c -> t p c", p=128)
        wo_r = woT_d[:].rearrange("(t p) c -> t p c", p=128)

        def xdma(i, eng):
            eng.dma_start(xraw[:, i * 512:(i + 1) * 512], xb_r[i])

        # sync queue: x0,x2, wk, biases, x4,x6, wq, x8..x14
        # scalar queue: x1,x3, wv, bv, x5,x7, wo, x9..x15
        xdma(0, nc.sync); xdma(2, nc.sync)
        xdma(1, nc.scalar); xdma(3, nc.scalar)
        for t in range(NE):
            nc.sync.dma_start(wk_sb[:, t * 256:(t + 1) * 256], wk_r[t])
            nc.scalar.dma_start(wv_sb[:, t * 256:(t + 1) * 256], wv_r[t])
        for hp in range(2):
            nc.sync.dma_start(bq_sb[hp][:], bq_d[hp])
            nc.sync.dma_start(bk_sb[hp][:], bk_d[hp])
        nc.scalar.dma_start(bv_sb[:], bv_d[:])
        for i in range(4, min(8, nS)):
            xdma(i, nc.sync if i % 2 == 0 else nc.scalar)
        for t in range(NE):
            nc.sync.dma_start(wq_sb[:, t * 256:(t + 1) * 256], wq_r[t])
        for t in range(2):
            nc.scalar.dma_start(wo_sb[:, t * E:(t + 1) * E], wo_r[t])
        for i in range(8, nS):
            xdma(i, nc.sync if i % 2 == 0 else nc.scalar)

        # ---- consts ----
        ident = const.tile([128, 128], BF16)
        make_identity(nc, ident[:])
        ones_pf = const.tile([128, 128], F32)
        nc.vector.memset(ones_pf[:], 1.0)
        ones64 = const.tile([1, 64], BF16)
        nc.vector.tensor_copy(ones64[:], ones_pf[0:1, 0:64])

        # ---- big SBUF tensors ----
        xT_sb = big.tile([128, NE * S], BF16)      # (e%128, et*S + s)
        qT_sb = big.tile([128, 2 * S], BF16)       # (hq*64+d, hp*S + s)
        kT_sb = big.tile([128, 2 * S], BF16)       # (hq*64+d, hp*S + s)
        v_sb = big.tile([128, HPC * nS * 65], BF16)  # (s%128, h*(nS*65)+c*65+d)
        aoT_sb = big.tile([128, 2 * S], BF16)      # (hq*64+d, hp*S + s)

        xT_r = xT_sb[:].rearrange("p (t s) -> p t s", t=NE)
        v_view = v_sb[:].rearrange("p (h s) -> p h s", h=HPC)
        # ones column (col 64 of each chunk slot)
        nc.vector.memset(
            v_sb[:].rearrange("p (g d) -> p g d", d=65)[:, :, 64:65], 1.0)

        # ---- emitters ----
        def tr_chunk(i):
            """x chunk i: cast f32->bf16, PE transpose, copy to xT."""
            xc = xcpool.tile([128, 512], BF16, tag="xc")
            if i < 8:
                nc.scalar.activation(xc[:], xraw[:, i * 512:(i + 1) * 512], COPY)
            else:
                nc.vector.tensor_copy(xc[:], xraw[:, i * 512:(i + 1) * 512])
            tp = ps_util.tile([128, 512], BF16, tag="util")
            for et in range(NE):
                nc.tensor.transpose(tp[:, et * 128:(et + 1) * 128],
                                    xc[:, et * 128:(et + 1) * 128], ident[:])
            nc.vector.tensor_copy(
                xT_r[:, :, i * 128:(i + 1) * 128],
                tp[:].rearrange("p (t s) -> p t s", t=NE))

        def qk_group(hp, which, j):
            """Project q or k for head pair hp, s-range [512j, 512j+512)."""
            w_sb = wq_sb if which == "q" else wk_sb
            pp = ps_util.tile([128, 512], F32, tag="util")
            for et in range(NE):
                nc.tensor.matmul(
                    pp[:],
                    lhsT=w_sb[:, et * 256 + hp * 128: et * 256 + (hp + 1) * 128],
                    rhs=xT_sb[:, et * S + j * 512: et * S + (j + 1) * 512],
                    start=(et == 0), stop=(et == NE - 1))
            dst = (qT_sb if which == "q" else kT_sb)
            sl = dst[:, hp * S + j * 512: hp * S + (j + 1) * 512]
            if which == "q":
                # fold the 1/sqrt(D) softmax scale into Q (bq pre-scaled)
                nc.vector.tensor_scalar(
                    sl, pp[:], 0.125, bq_sb[hp][:],
                    op0=mybir.AluOpType.mult, op1=mybir.AluOpType.add)
            else:
                nc.vector.tensor_scalar_add(sl, pp[:], bk_sb[hp][:])

        def v_group(i):
            vp = ps_util.tile([128, 512], F32, tag="util")
            vps = vp[:, 0:256]
            for et in range(NE):
                nc.tensor.matmul(
                    vps,
                    lhsT=xT_sb[:, et * S + i * 128: et * S + (i + 1) * 128],
                    rhs=wv_sb[:, et * 256:(et + 1) * 256],
                    start=(et == 0), stop=(et == NE - 1))
            nc.vector.tensor_add(
                v_view[:, :, i * 65: i * 65 + 64],
                vps.rearrange("p (h d) -> p h d", h=HPC),
                bv_sb[:].rearrange("p (h d) -> p h d", h=HPC))

        def out_proj_chunk(ct, c):
            """Partial out-proj for s-chunk c over head pair ct's 128 dims."""
            yps = ps_util.tile([128, 512], F32, tag="util")
            nc.tensor.matmul(
                yps[:],
                lhsT=aoT_sb[:, ct * S + c * 128: ct * S + (c + 1) * 128],
                rhs=wo_sb[:, ct * E:(ct + 1) * E],
                start=True, stop=True)
            ys = ypool.tile([128, E], BF16, tag="ys")
            nc.vector.tensor_copy(ys[:], yps[:])
            nc.sync.dma_start(yp_rs[ct][c], ys[:])

        # ---- filler plan (fancy path): global iter -> list of emitters ----
        plan = defaultdict(list)
        if fancy:
            # block order: (half, hp, hq); 16 iters each; 128 global iters
            plan[0] = [lambda: tr_chunk(8), lambda: v_group(0)]
            plan[1] = [lambda: tr_chunk(9), lambda: v_group(1)]
            plan[2] = [lambda: tr_chunk(10), lambda: v_group(2)]
            plan[3] = [lambda: qk_group(0, "k", 1), lambda: v_group(3)]
            plan[4] = [lambda: tr_chunk(11), lambda: v_group(4)]
            plan[5] = [lambda: tr_chunk(12), lambda: v_group(5)]
            plan[6] = [lambda: qk_group(0, "k", 2), lambda: v_group(6)]
            plan[7] = [lambda: tr_chunk(13), lambda: v_group(7)]
            plan[8] = [lambda: tr_chunk(14), lambda: v_group(8)]
            plan[9] = [lambda: tr_chunk(15), lambda: v_group(9)]
            plan[10] = [lambda: qk_group(0, "k", 3), lambda: v_group(10)]
            for i in range(11, 16):
                plan[i] = [lambda i=i: v_group(i)]
            # block 1 (iters 16-31): hp1 projections + remaining q
            plan[16] = [lambda: qk_group(1, "k", 0)]
            plan[17] = [lambda: qk_group(1, "k", 1)]
            plan[18] = [lambda: qk_group(1, "q", 0)]
            plan[19] = [lambda: qk_group(1, "q", 1)]
            plan[20] = [lambda: qk_group(1, "k", 2)]
            plan[21] = [lambda: qk_group(1, "k", 3)]
            plan[22] = [lambda: qk_group(0, "q", 2)]
            plan[23] = [lambda: qk_group(0, "q", 3)]
            plan[24] = [lambda: qk_group(1, "q", 2)]
            plan[25] = [lambda: qk_group(1, "q", 3)]
            # out-proj: (ct0, 0-7) once half0/hp0 is normalized (norm tails
            # land at git 35,36); similarly (ct1, 0-7) after 64 and
            # (ct0, 8-15) after 96; (ct1, 8-15) is the tail.
            for c in range(8):
                plan[37 + 2 * c].append(lambda c=c: out_proj_chunk(0, c))
                plan[69 + 2 * c].append(lambda c=c: out_proj_chunk(1, c))
                plan[101 + 2 * c].append(lambda c=c: out_proj_chunk(0, 8 + c))

        # ---- pre-attention minimal prefix ----
        n_pre = min(8, nS) if fancy else nS
        for i in range(n_pre):
            tr_chunk(i)
        qk_group(0, "k", 0)
        for j in range(min(JW, nSq)):
            qk_group(0, "q", j)
        if not fancy:
            # strict order for small-S sim: everything up front
            for j in range(1, nSq):
                qk_group(0, "k", j)
            for j in range(JW, nSq):
                qk_group(0, "q", j)
            for hp in (1,):
                for j in range(nSq):
                    qk_group(hp, "k", j)
                    qk_group(hp, "q", j)
            for i in range(nS):
                v_group(i)

        # ---- attention: halves outer, AV delayed one iteration ----
        git = 0          # global iteration counter
        pending = []     # (acc, pt, h, i, is_last, norm_ctx)
        norm_q = []      # deferred normalize-tail stages (bc matmul + mul)

        def emit_av(item):
            acc, pt, h, i, last, nctx = item
            for j2 in range(JW):
                nc.tensor.matmul(
                    acc[:, j2 * 512:(j2 + 1) * 512],
                    lhsT=v_sb[:, h * nS * 65 + i * 65: h * nS * 65 + i * 65 + 65],
                    rhs=pt[:, j2 * 512:(j2 + 1) * 512],
                    start=(i == 0), stop=last,
                    skip_group_check=True)
            if last:
                emit_norm_head(acc, nctx)

        def emit_norm_head(acc, nctx):
            """DVE part of softmax-normalize; queues the PE/mul tail."""
            hp, hq, half = nctx
            r0, r1 = 64 * hq, 64 * hq + 64
            ssb = smpool.tile([1, W], F32, tag="ssb")
            nc.vector.tensor_copy(ssb[:], acc[64:65, :])
            aou = bcpool.tile([64, W], F32, tag="aou")
            nc.vector.tensor_copy(aou[:], acc[0:64, :])
            rsb = smpool.tile([1, W], F32, tag="rsb")
            nc.vector.reciprocal_approx_fast(rsb[:], ssb[:])
            rf = smpool.tile([1, W], BF16, tag="rf")
            nc.vector.tensor_copy(rf[:], rsb[:])

            def tail(j2):
                bc = ps_util.tile([64, 512], F32, tag="util")
                nc.tensor.matmul(bc[:], lhsT=ones64[:],
                                 rhs=rf[:, j2 * 512:(j2 + 1) * 512],
                                 start=True, stop=True)
                nc.vector.tensor_mul(
                    aoT_sb[r0:r1,
                           hp * S + half * W + j2 * 512:
                           hp * S + half * W + (j2 + 1) * 512],
                    aou[:, j2 * 512:(j2 + 1) * 512], bc[:])
            for j2 in range(JW):
                # leave the DVE reciprocal chain >=3 iterations of headroom
                # before the PE bc matmul needs its result
                norm_q.append((git + 3 + j2, lambda j2=j2: tail(j2)))

        for half in range(nH2):
            for hp in range(2):
                for hq in range(2):
                    h = 2 * hp + hq
                    r0, r1 = 64 * hq, 64 * hq + 64
                    acc = ps_acc.tile([65, W], F32, name=f"acc{half}_{h}", tag="acc")
                    for i in range(nS):
                        sc = ps_sc.tile([128, W], F32, tag="sc")
                        for j2 in range(JW):
                            nc.tensor.matmul(
                                sc[:, j2 * 512:(j2 + 1) * 512],
                                lhsT=kT_sb[r0:r1,
                                           hp * S + i * 128:
                                           hp * S + (i + 1) * 128],
                                rhs=qT_sb[r0:r1,
                                          hp * S + half * W + j2 * 512:
                                          hp * S + half * W + (j2 + 1) * 512],
                                start=True, stop=True)
                        pt = ptpool.tile([128, W], BF16, tag="pt")
                        nc.scalar.activation(pt[:], sc[:], EXP)
                        pending.append((acc, pt, h, i, i == nS - 1,
                                        (hp, hq, half)))
                        # keep exactly one AV pending (one-iteration delay)
                        while len(pending) > 1:
                            emit_av(pending.pop(0))
                        while norm_q and norm_q[0][0] <= git:
                            norm_q.pop(0)[1]()
                        for fn in plan.get(git, ()):
                            fn()
                        git += 1

        while pending:
            emit_av(pending.pop(0))
        while norm_q:
            norm_q.pop(0)[1]()

        # ---- tail out-proj ----
        if fancy:
            for c in range(8, 16):
                out_proj_chunk(1, c)
        else:
            for ct in range(2):
                for c in range(nS):
                    out_proj_chunk(ct, c)

    nc.compile()
    return nc


def make_in_maps(x, w_qkv, b_qkv, w_out):
    """Build the 8 per-core input dicts from full inputs."""
    in_maps = []
    for c in range(8):
        b, hg = c // 2, c % 2
        r0 = hg * 256
        wq = w_qkv[r0:r0 + 256, :]
        wk = w_qkv[512 + r0:512 + r0 + 256, :]
        wv = w_qkv[1024 + r0:1024 + r0 + 256, :]
        in_maps.append({
            "xb": np.ascontiguousarray(x[b]),
            "wqT": np.ascontiguousarray(wq.T.astype(ml_dtypes.bfloat16)),
            "wkT": np.ascontiguousarray(wk.T.astype(ml_dtypes.bfloat16)),
            "wvT": np.ascontiguousarray(wv.T.astype(ml_dtypes.bfloat16)),
            "woT": np.ascontiguousarray(w_out[:, r0:r0 + 256].T.astype(ml_dtypes.bfloat16)),
            "bq": np.ascontiguousarray(b_qkv[r0:r0 + 256].reshape(2, 128, 1) * 0.125),
            "bk": np.ascontiguousarray(b_qkv[512 + r0:512 + r0 + 256].reshape(2, 128, 1)),
            "bv": np.ascontiguousarray(
                np.tile(b_qkv[1024 + r0:1024 + r0 + 256][None, :], (128, 1))),
        })
    return in_maps


_cached_nc = None
last_exec_time_ns = None
last_result = None


def kernel(x, w_qkv, b_qkv, w_out, b_out):
    global _cached_nc, last_exec_time_ns, last_result
    from concourse.bass_utils import run_bass_kernel_spmd

    x = np.asarray(x, dtype=np.float32)
    w_qkv = np.asarray(w_qkv, dtype=np.float32)
    b_qkv = np.asarray(b_qkv, dtype=np.float32)
    w_out = np.asarray(w_out, dtype=np.float32)
    b_out = np.asarray(b_out, dtype=np.float32)
    B, S, _ = x.shape

    if _cached_nc is None:
        _cached_nc = build(S)
    nc = _cached_nc

    in_maps = make_in_maps(x, w_qkv, b_qkv, w_out)
    trace = bool(os.environ.get("BASS_KERNEL_TRACE"))
    r = run_bass_kernel_spmd(nc, in_maps, core_ids=list(range(8)), trace=trace)
    last_exec_time_ns = r.exec_time_ns
    last_result = r

    y = np.empty((B, S, E), dtype=np.float32)
    for b in range(B):
        y[b] = (r.results[2 * b]["yp0"].astype(np.float32)
                + r.results[2 * b]["yp1"].astype(np.float32)
                + r.results[2 * b + 1]["yp0"].astype(np.float32)
                + r.results[2 * b + 1]["yp1"].astype(np.float32) + b_out)
    return y
